# revision 29
# baseline (speedup 1.0000x reference)
"""Trainium2 Bass kernel for nn_DiscriminativeAlignmentLoss.

loss = 0.5*(CE_row + CE_col) over logits = -dist/T,
dist = (1/sqrt(c)) * arccosh(c*(v_time*t_time - v.t))   (Lorentz pairwise)

Strategy (8 cores, data parallel over v rows):
  - Each core owns 1024 v rows and all 8192 t rows. The Lorentz inner
    product is one PSUM accumulation: the 768 feature dims as fp8-e4m3
    DoubleRow matmuls (K=256 per instruction), plus a small bf16 K=4
    matmul carrying the (hi, lo) bf16 split of the v_time*t_time product
    (which needs much more precision than the feature dot).
  - arccosh(x) = ln(2x) - 1/(4x^2) - ...; for this data x >= ~570 so
    ln(2x) is exact to ~1e-11. Chain on ScalarE over 2048-wide chunks:
    Ln in place in PSUM (scale=-c), then Exp (scale=-k, constant bias
    -(S_core + k*ln2), so e = exp(logit - S_core)). Both functions live
    in one ACT table set (the greedy set picker is patched below).
  - Exp's accum_out yields row partial sums for free (fp32, pre-cast).
    Column partials are plain partition-wise sums: VectorE accumulates
    e chunks into a [128, 8192] fp32 buffer; the 128-row reduction and
    all shift/log arithmetic happen on host in fp64.
"""

import numpy as np
import ml_dtypes

import concourse.bass as bass  # noqa: F401  (registers AP machinery)
import concourse.tile as tile
from concourse import bacc, mybir
from concourse import hw_specs as _hw_specs
from concourse.bass_utils import run_bass_kernel_spmd

# The act-table insertion pass resolves each activation to the FIRST set
# containing its function: Exp -> exp_and_others, Ln -> natural_log. With
# Ln/Exp alternating per tile that means an ACT_TABLE_LOAD (~1.3us) before
# nearly every ACTIVATE (~162us/core wasted, measured). Restrict Ln/Exp to
# the combined set (same names/order, so set ids stay canonical) so the
# fixpoint hoists a single load.
_orig_get_activation_tables = _hw_specs.get_activation_tables


def _patched_get_activation_tables(arch):
    tables = _orig_get_activation_tables(arch)
    drop = {mybir.ActivationFunctionType.Ln, mybir.ActivationFunctionType.Exp}
    return {
        name: (funcs if name == "natural_log_exp_and_others" else funcs - drop)
        for name, funcs in tables.items()
    }


bacc.get_activation_tables = _patched_get_activation_tables

N = 8192
D = 768
NCORES = 8
R = N // NCORES  # 1024 rows per core
MT = 8  # 128-row m-tiles per core
NQ = 4  # 2048-column chunks
KT = 6  # 128-row K subtiles (768 = 6*128)
KAUG = 4  # augmented K rows (hi/lo split of the time product)
TEMPERATURE = 0.07
EPS = 1e-6
LN2 = float(np.log(2.0))
bf16 = ml_dtypes.bfloat16
fp8 = ml_dtypes.float8_e4m3
dt = mybir.dt

_program_cache = {}


def _build_program(c: float):
    """Build + compile the per-core Bass program (same on all 8 cores)."""
    k_eff = (1.0 / c) ** 0.5 / TEMPERATURE
    nc = bacc.Bacc(
        "TRN2",
        target_bir_lowering=False,
        debug=False,
        enable_asserts=False,
        num_devices=NCORES,
    )

    vt8_d = nc.dram_tensor("vt8", [128, KT, R], dt.float8e4, kind="ExternalInput")
    # strip-major so each strip's DMA reads 12KB-contiguous rows
    tt8_d = nc.dram_tensor(
        "tt8", [NQ, 128, KT, 2048], dt.float8e4, kind="ExternalInput"
    )
    vtail_d = nc.dram_tensor("vtail", [KAUG, R], dt.bfloat16, kind="ExternalInput")
    ttail_d = nc.dram_tensor("ttail", [KAUG, N], dt.bfloat16, kind="ExternalInput")
    bias_d = nc.dram_tensor("bias", [128, 1], dt.float32, kind="ExternalInput")
    rowparts_d = nc.dram_tensor(
        "rowparts", [128, MT * NQ], dt.float32, kind="ExternalOutput"
    )
    colsum_d = nc.dram_tensor("colsum", [128, N], dt.float32, kind="ExternalOutput")

    DR = mybir.MatmulPerfMode.DoubleRow

    with tile.TileContext(nc) as tc:
        with (
            tc.tile_pool(name="consts", bufs=1) as consts,
            tc.tile_pool(name="epool", bufs=3) as epool,
            tc.tile_pool(name="mmps", bufs=2, space="PSUM") as mmps,
        ):
            # per-strip tiles so chunk-nq compute only RAW-depends on its
            # own strip's DMA
            tt8_t = [
                consts.tile([128, KT, 2048], dt.float8e4, name=f"tt8_{s}")
                for s in range(NQ)
            ]
            tt_tail = [
                consts.tile([KAUG, 2048], dt.bfloat16, name=f"tt_tail{s}")
                for s in range(NQ)
            ]
            vt8_t = consts.tile([128, KT, R], dt.float8e4, name="vt8_t")
            vt_tail = consts.tile([KAUG, R], dt.bfloat16, name="vt_tail")
            bias_t = consts.tile([128, 1], dt.float32, name="bias_t")
            rowparts_t = consts.tile([128, MT * NQ], dt.float32, name="rowparts_t")
            colaccP = consts.tile([128, N], dt.float32, name="colaccP")

            # Critical-path tensors on the two hardware DGE queues (sync,
            # scalar) -- the gpsimd queue is software-descriptor (slow) and
            # only gets the small tail strips.
            nc.sync.dma_start(out=vt8_t, in_=vt8_d[:, :, :])
            nc.scalar.dma_start(out=vt_tail, in_=vtail_d[:, :])
            nc.scalar.dma_start(out=bias_t, in_=bias_d[:, :])
            for s in range(NQ):
                cs = slice(s * 2048, (s + 1) * 2048)
                nc.sync.dma_start(out=tt8_t[s][:, :3, :], in_=tt8_d[s, :, :3, :])
                nc.scalar.dma_start(out=tt8_t[s][:, 3:, :], in_=tt8_d[s, :, 3:, :])
                nc.gpsimd.dma_start(out=tt_tail[s], in_=ttail_d[:, cs])

            # preload the Ln/Exp ACT table set during the DMA prologue so the
            # first real activation doesn't pay the ~1.3us table load (which
            # would stall the PE pipeline fill long enough to re-throttle HAM)
            scratch = consts.tile([128, 1], dt.float32, name="scratch")
            nc.vector.memset(scratch[:, :], 1.0)
            nc.scalar.activation(
                scratch[:, :], scratch[:, :], mybir.ActivationFunctionType.Ln
            )

            # Warmup matmuls on zeroed scratch: keep TensorE busy through the
            # DMA prologue + pipeline fill so the HAM clock gate reaches (and
            # keeps) 2.4 GHz before the real matmul stream starts.
            warm_w = consts.tile([128, 64], dt.bfloat16, name="warm_w")
            nc.vector.memset(warm_w[:, :], 0.0)
            pm_warm = mmps.tile([128, 2048], dt.float32, name="pmw", tag="pm")
            for _ in range(120):
                nc.tensor.matmul(
                    pm_warm[:1, :64], warm_w[:, 0:1], warm_w[:, :], start=True,
                    stop=True,
                )

            # zero the column accumulator (DVE memset)
            nc.vector.memset(colaccP[:, :], 0.0)

            for nq in range(NQ):
                for m in range(MT):
                    ms = slice(m * 128, (m + 1) * 128)
                    pm = mmps.tile([128, 2048], dt.float32, name="pm", tag="pm")
                    for g in range(4):
                        gs = slice(g * 512, (g + 1) * 512)
                        for kp in range(KT // 2):
                            sp = slice(2 * kp, 2 * kp + 2)
                            nc.tensor.matmul(
                                pm[:, gs],
                                vt8_t[:, sp, ms],
                                tt8_t[nq][:, sp, gs],
                                start=(kp == 0),
                                stop=False,
                                perf_mode=DR,
                            )
                        nc.tensor.matmul(
                            pm[:, gs],
                            vt_tail[:, ms],
                            tt_tail[nq][:, gs],
                            start=False,
                            stop=True,
                        )
                    # ln in place in PSUM, then exp reads PSUM directly
                    nc.scalar.activation(
                        pm[:, :],
                        pm[:, :],
                        mybir.ActivationFunctionType.Ln,
                        scale=float(-c),
                    )
                    et = epool.tile([128, 2048], dt.bfloat16, name="et", tag="et")
                    idx = m * NQ + nq
                    nc.scalar.activation(
                        et[:, :],
                        pm[:, :],
                        mybir.ActivationFunctionType.Exp,
                        bias=bias_t[:, 0:1],
                        scale=float(-k_eff),
                        accum_out=rowparts_t[:, idx : idx + 1],
                    )
                    cs = slice(nq * 2048, (nq + 1) * 2048)
                    nc.vector.tensor_add(colaccP[:, cs], colaccP[:, cs], et[:, :])
                cs = slice(nq * 2048, (nq + 1) * 2048)
                nc.sync.dma_start(out=colsum_d[:, cs], in_=colaccP[:, cs])

            nc.sync.dma_start(out=rowparts_d[:, :], in_=rowparts_t)

    nc.compile()
    return nc


def _host_prep(v, t, c_val):
    """fp64 host-side constants: diag logits (shifts), fp8/bf16 operands."""
    v64 = np.asarray(v, np.float64)
    t64 = np.asarray(t, np.float64)
    inv_c = 1.0 / c_val
    k_eff = inv_c**0.5 / TEMPERATURE

    v_time = np.sqrt(inv_c + np.einsum("nd,nd->n", v64, v64))
    t_time = np.sqrt(inv_c + np.einsum("nd,nd->n", t64, t64))
    diag_dot = np.einsum("nd,nd->n", v64, t64)
    diag_arg = np.maximum(c_val * (v_time * t_time - diag_dot), 1.0 + EPS)
    a = -k_eff * np.arccosh(diag_arg)  # diag logits, used as row/col shifts

    # [p, subtile, col] layout: element [p, s, j] = x[col j, feature s*128+p]
    v8 = np.asarray(v, np.float32).astype(fp8)
    t8 = np.asarray(t, np.float32).astype(fp8)
    vt8 = np.ascontiguousarray(v8.T.reshape(KT, 128, N).transpose(1, 0, 2))
    tt8_full = t8.T.reshape(KT, 128, N).transpose(1, 0, 2)  # [p, s, j]
    # strip-major [strip, p, subtile, j-within-strip]
    tt8 = np.ascontiguousarray(
        tt8_full.reshape(128, KT, NQ, 2048).transpose(2, 0, 1, 3)
    )

    vth = v_time.astype(np.float32).astype(bf16)
    vtl = (v_time.astype(np.float32) - vth.astype(np.float32)).astype(bf16)
    tth = t_time.astype(np.float32).astype(bf16)
    ttl = (t_time.astype(np.float32) - tth.astype(np.float32)).astype(bf16)
    vtail = np.stack([vth, vtl, vth, vtl])  # [4, N]
    ttail = np.stack([-tth, -tth, -ttl, -ttl])  # [4, N]
    return a, k_eff, vt8, tt8, vtail, ttail


last_run_info = {}


def kernel(v_hyp, t_hyp, c, _trace=False):
    c_val = float(np.asarray(c))
    a, k_eff, vt8, tt8, vtail, ttail = _host_prep(v_hyp, t_hyp, c_val)

    key = c_val
    if key not in _program_cache:
        _program_cache[key] = _build_program(c_val)
    nc = _program_cache[key]

    S = np.array([a[k * R : (k + 1) * R].max() for k in range(NCORES)])
    in_maps = []
    for k in range(NCORES):
        rows = slice(k * R, (k + 1) * R)
        bias_mat = np.full((128, 1), -(S[k] + k_eff * LN2), np.float32)
        in_maps.append(
            {
                "vt8": np.ascontiguousarray(vt8[:, :, rows]),
                "tt8": tt8,
                "vtail": np.ascontiguousarray(vtail[:, rows]),
                "ttail": ttail,
                "bias": bias_mat,
            }
        )

    # Rare first-execution flake has been observed to return garbage once;
    # outputs are cheap to validate (rowparts must be finite and positive),
    # so retry a couple of times if that happens.
    for attempt in range(3):
        res = run_bass_kernel_spmd(nc, in_maps, list(range(NCORES)), trace=_trace)
        last_run_info["results"] = res
        results = res.results
        ok = all(
            np.all(np.isfinite(results[k]["rowparts"]))
            and np.all(results[k]["rowparts"] > 0)
            and np.all(np.isfinite(results[k]["colsum"]))
            for k in range(NCORES)
        )
        if ok:
            break

    # rowsum'_i = sum_j exp(x_ij - S_k); ln(sum_j exp(x_ij - a_i))
    #           = ln(rowsum'_i) + (S_k - a_i)
    rowLSE_minus_a = np.empty(N, np.float64)
    colsum_parts = np.empty((NCORES, N), np.float64)
    for k in range(NCORES):
        rp = results[k]["rowparts"].astype(np.float64)  # [128, MT*NQ]
        rp_pm = rp.reshape(128, MT, NQ).sum(axis=2)  # [p, m]
        rows = slice(k * R, (k + 1) * R)
        rowLSE_minus_a[rows] = np.log(rp_pm.T.reshape(R)) + (S[k] - a[rows])
        colsum_parts[k] = results[k]["colsum"].astype(np.float64).sum(axis=0)

    loss_v2t = np.mean(rowLSE_minus_a)
    M0 = S.max()
    col = (colsum_parts * np.exp(S - M0)[:, None]).sum(axis=0)
    loss_t2v = np.mean(np.log(col) + M0 - a)
    return np.asarray(0.5 * (loss_v2t + loss_t2v), dtype=np.float32)


# revision 30
# speedup vs baseline: 1.1907x; 1.1907x over previous
"""Trainium2 Bass kernel for nn_DiscriminativeAlignmentLoss.

loss = 0.5*(CE_row + CE_col) over logits = -dist/T,
dist = (1/sqrt(c)) * arccosh(c*(v_time*t_time - v.t))   (Lorentz pairwise)

Strategy (8 cores, data parallel over v rows):
  - Each core owns 1024 v rows and all 8192 t rows. The Lorentz inner
    product is one PSUM accumulation: the 768 feature dims as fp8-e4m3
    DoubleRow matmuls (K=256 per instruction), plus a small bf16 K=4
    matmul carrying the (hi, lo) bf16 split of the v_time*t_time product
    (which needs much more precision than the feature dot).
  - arccosh(x) = ln(2x) - 1/(4x^2) - ...; for this data x >= ~570 so
    ln(2x) is exact to ~1e-11. Chain on ScalarE over 2048-wide chunks:
    Ln in place in PSUM (scale=-c), then Exp (scale=-k, constant bias
    -(S_core + k*ln2), so e = exp(logit - S_core)). Both functions live
    in one ACT table set (the greedy set picker is patched below).
  - Exp's accum_out yields row partial sums for free (fp32, pre-cast).
    Column partials are plain partition-wise sums: VectorE accumulates
    e chunks into a [128, 8192] fp32 buffer; the 128-row reduction and
    all shift/log arithmetic happen on host in fp64.
"""

import numpy as np
import ml_dtypes

import concourse.bass as bass  # noqa: F401  (registers AP machinery)
import concourse.tile as tile
from concourse import bacc, mybir
from concourse import hw_specs as _hw_specs
from concourse.bass_utils import run_bass_kernel_spmd

# The act-table insertion pass resolves each activation to the FIRST set
# containing its function: Exp -> exp_and_others, Ln -> natural_log. With
# Ln/Exp alternating per tile that means an ACT_TABLE_LOAD (~1.3us) before
# nearly every ACTIVATE (~162us/core wasted, measured). Restrict Ln/Exp to
# the combined set (same names/order, so set ids stay canonical) so the
# fixpoint hoists a single load.
_orig_get_activation_tables = _hw_specs.get_activation_tables


def _patched_get_activation_tables(arch):
    tables = _orig_get_activation_tables(arch)
    drop = {mybir.ActivationFunctionType.Ln, mybir.ActivationFunctionType.Exp}
    return {
        name: (funcs if name == "natural_log_exp_and_others" else funcs - drop)
        for name, funcs in tables.items()
    }


bacc.get_activation_tables = _patched_get_activation_tables

N = 8192
D = 768
NCORES = 8
R = N // NCORES  # 1024 rows per core
MT = 8  # 128-row m-tiles per core
NQ = 4  # 2048-column chunks
KT = 6  # 128-row K subtiles (768 = 6*128)
KAUG = 4  # augmented K rows (hi/lo split of the time product)
TEMPERATURE = 0.07
EPS = 1e-6
LN2 = float(np.log(2.0))
bf16 = ml_dtypes.bfloat16
fp8 = ml_dtypes.float8_e4m3
dt = mybir.dt

_program_cache = {}


def _build_program(c: float):
    """Build + compile the per-core Bass program (same on all 8 cores)."""
    k_eff = (1.0 / c) ** 0.5 / TEMPERATURE
    nc = bacc.Bacc(
        "TRN2",
        target_bir_lowering=False,
        debug=False,
        enable_asserts=False,
        num_devices=NCORES,
    )

    vt8_d = nc.dram_tensor("vt8", [128, KT, R], dt.float8e4, kind="ExternalInput")
    # strip-major so each strip's DMA reads 12KB-contiguous rows
    tt8_d = nc.dram_tensor(
        "tt8", [NQ, 128, KT, 2048], dt.float8e4, kind="ExternalInput"
    )
    vtail_d = nc.dram_tensor("vtail", [KAUG, R], dt.bfloat16, kind="ExternalInput")
    ttail_d = nc.dram_tensor("ttail", [KAUG, N], dt.bfloat16, kind="ExternalInput")
    bias_d = nc.dram_tensor("bias", [128, 1], dt.float32, kind="ExternalInput")
    rowparts_d = nc.dram_tensor(
        "rowparts", [128, MT * NQ], dt.float32, kind="ExternalOutput"
    )
    colsum_d = nc.dram_tensor("colsum", [128, N], dt.float32, kind="ExternalOutput")

    DR = mybir.MatmulPerfMode.DoubleRow

    with tile.TileContext(nc) as tc:
        with (
            tc.tile_pool(name="consts", bufs=1) as consts,
            tc.tile_pool(name="epool", bufs=3) as epool,
            tc.tile_pool(name="mmps", bufs=2, space="PSUM") as mmps,
        ):
            # per-strip tiles so chunk-nq compute only RAW-depends on its
            # own strip's DMA
            tt8_t = [
                consts.tile([128, KT, 2048], dt.float8e4, name=f"tt8_{s}")
                for s in range(NQ)
            ]
            tt_tail = [
                consts.tile([KAUG, 2048], dt.bfloat16, name=f"tt_tail{s}")
                for s in range(NQ)
            ]
            vt8_t = consts.tile([128, KT, R], dt.float8e4, name="vt8_t")
            vt_tail = consts.tile([KAUG, R], dt.bfloat16, name="vt_tail")
            bias_t = consts.tile([128, 1], dt.float32, name="bias_t")
            rowparts_t = consts.tile([128, MT * NQ], dt.float32, name="rowparts_t")
            colaccP = consts.tile([128, N], dt.float32, name="colaccP")

            # Critical-path tensors on the two hardware DGE queues (sync,
            # scalar) -- the gpsimd queue is software-descriptor (slow) and
            # only gets the small tail strips.
            nc.sync.dma_start(out=vt8_t, in_=vt8_d[:, :, :])
            nc.scalar.dma_start(out=vt_tail, in_=vtail_d[:, :])
            nc.scalar.dma_start(out=bias_t, in_=bias_d[:, :])
            for s in range(NQ):
                cs = slice(s * 2048, (s + 1) * 2048)
                nc.sync.dma_start(out=tt8_t[s][:, :3, :], in_=tt8_d[s, :, :3, :])
                nc.scalar.dma_start(out=tt8_t[s][:, 3:, :], in_=tt8_d[s, :, 3:, :])
                nc.gpsimd.dma_start(out=tt_tail[s], in_=ttail_d[:, cs])

            # preload the Ln/Exp ACT table set during the DMA prologue so the
            # first real activation doesn't pay the ~1.3us table load (which
            # would stall the PE pipeline fill long enough to re-throttle HAM)
            scratch = consts.tile([128, 1], dt.float32, name="scratch")
            nc.vector.memset(scratch[:, :], 1.0)
            nc.scalar.activation(
                scratch[:, :], scratch[:, :], mybir.ActivationFunctionType.Ln
            )

            # zero the column accumulator (DVE memset)
            nc.vector.memset(colaccP[:, :], 0.0)

            for nq in range(NQ):
                for m in range(MT):
                    ms = slice(m * 128, (m + 1) * 128)
                    pm = mmps.tile([128, 2048], dt.float32, name="pm", tag="pm")
                    for g in range(4):
                        gs = slice(g * 512, (g + 1) * 512)
                        for kp in range(KT // 2):
                            sp = slice(2 * kp, 2 * kp + 2)
                            nc.tensor.matmul(
                                pm[:, gs],
                                vt8_t[:, sp, ms],
                                tt8_t[nq][:, sp, gs],
                                start=(kp == 0),
                                stop=False,
                                perf_mode=DR,
                            )
                        nc.tensor.matmul(
                            pm[:, gs],
                            vt_tail[:, ms],
                            tt_tail[nq][:, gs],
                            start=False,
                            stop=True,
                        )
                    # ln in place in PSUM, then exp reads PSUM directly
                    nc.scalar.activation(
                        pm[:, :],
                        pm[:, :],
                        mybir.ActivationFunctionType.Ln,
                        scale=float(-c),
                    )
                    et = epool.tile([128, 2048], dt.bfloat16, name="et", tag="et")
                    idx = m * NQ + nq
                    nc.scalar.activation(
                        et[:, :],
                        pm[:, :],
                        mybir.ActivationFunctionType.Exp,
                        bias=bias_t[:, 0:1],
                        scale=float(-k_eff),
                        accum_out=rowparts_t[:, idx : idx + 1],
                    )
                    cs = slice(nq * 2048, (nq + 1) * 2048)
                    nc.vector.tensor_add(colaccP[:, cs], colaccP[:, cs], et[:, :])
                cs = slice(nq * 2048, (nq + 1) * 2048)
                nc.sync.dma_start(out=colsum_d[:, cs], in_=colaccP[:, cs])

            nc.sync.dma_start(out=rowparts_d[:, :], in_=rowparts_t)

    nc.compile()
    return nc


def _host_prep(v, t, c_val):
    """fp64 host-side constants: diag logits (shifts), fp8/bf16 operands."""
    v64 = np.asarray(v, np.float64)
    t64 = np.asarray(t, np.float64)
    inv_c = 1.0 / c_val
    k_eff = inv_c**0.5 / TEMPERATURE

    v_time = np.sqrt(inv_c + np.einsum("nd,nd->n", v64, v64))
    t_time = np.sqrt(inv_c + np.einsum("nd,nd->n", t64, t64))
    diag_dot = np.einsum("nd,nd->n", v64, t64)
    diag_arg = np.maximum(c_val * (v_time * t_time - diag_dot), 1.0 + EPS)
    a = -k_eff * np.arccosh(diag_arg)  # diag logits, used as row/col shifts

    # [p, subtile, col] layout: element [p, s, j] = x[col j, feature s*128+p]
    v8 = np.asarray(v, np.float32).astype(fp8)
    t8 = np.asarray(t, np.float32).astype(fp8)
    vt8 = np.ascontiguousarray(v8.T.reshape(KT, 128, N).transpose(1, 0, 2))
    tt8_full = t8.T.reshape(KT, 128, N).transpose(1, 0, 2)  # [p, s, j]
    # strip-major [strip, p, subtile, j-within-strip]
    tt8 = np.ascontiguousarray(
        tt8_full.reshape(128, KT, NQ, 2048).transpose(2, 0, 1, 3)
    )

    vth = v_time.astype(np.float32).astype(bf16)
    vtl = (v_time.astype(np.float32) - vth.astype(np.float32)).astype(bf16)
    tth = t_time.astype(np.float32).astype(bf16)
    ttl = (t_time.astype(np.float32) - tth.astype(np.float32)).astype(bf16)
    vtail = np.stack([vth, vtl, vth, vtl])  # [4, N]
    ttail = np.stack([-tth, -tth, -ttl, -ttl])  # [4, N]
    return a, k_eff, vt8, tt8, vtail, ttail


last_run_info = {}


def kernel(v_hyp, t_hyp, c, _trace=False):
    c_val = float(np.asarray(c))
    a, k_eff, vt8, tt8, vtail, ttail = _host_prep(v_hyp, t_hyp, c_val)

    key = c_val
    if key not in _program_cache:
        _program_cache[key] = _build_program(c_val)
    nc = _program_cache[key]

    S = np.array([a[k * R : (k + 1) * R].max() for k in range(NCORES)])
    in_maps = []
    for k in range(NCORES):
        rows = slice(k * R, (k + 1) * R)
        bias_mat = np.full((128, 1), -(S[k] + k_eff * LN2), np.float32)
        in_maps.append(
            {
                "vt8": np.ascontiguousarray(vt8[:, :, rows]),
                "tt8": tt8,
                "vtail": np.ascontiguousarray(vtail[:, rows]),
                "ttail": ttail,
                "bias": bias_mat,
            }
        )

    # Rare first-execution flake has been observed to return garbage once;
    # outputs are cheap to validate (rowparts must be finite and positive),
    # so retry a couple of times if that happens.
    for attempt in range(3):
        res = run_bass_kernel_spmd(nc, in_maps, list(range(NCORES)), trace=_trace)
        last_run_info["results"] = res
        results = res.results
        ok = all(
            np.all(np.isfinite(results[k]["rowparts"]))
            and np.all(results[k]["rowparts"] > 0)
            and np.all(np.isfinite(results[k]["colsum"]))
            for k in range(NCORES)
        )
        if ok:
            break

    # rowsum'_i = sum_j exp(x_ij - S_k); ln(sum_j exp(x_ij - a_i))
    #           = ln(rowsum'_i) + (S_k - a_i)
    rowLSE_minus_a = np.empty(N, np.float64)
    colsum_parts = np.empty((NCORES, N), np.float64)
    for k in range(NCORES):
        rp = results[k]["rowparts"].astype(np.float64)  # [128, MT*NQ]
        rp_pm = rp.reshape(128, MT, NQ).sum(axis=2)  # [p, m]
        rows = slice(k * R, (k + 1) * R)
        rowLSE_minus_a[rows] = np.log(rp_pm.T.reshape(R)) + (S[k] - a[rows])
        colsum_parts[k] = results[k]["colsum"].astype(np.float64).sum(axis=0)

    loss_v2t = np.mean(rowLSE_minus_a)
    M0 = S.max()
    col = (colsum_parts * np.exp(S - M0)[:, None]).sum(axis=0)
    loss_t2v = np.mean(np.log(col) + M0 - a)
    return np.asarray(0.5 * (loss_v2t + loss_t2v), dtype=np.float32)
